# revision 11
# baseline (speedup 1.0000x reference)
"""Trainium2 Bass kernel: nearest-centroid assignment (vq_codebook).

Computes, for each row of `feats` [N, 512]:
    f = normalize([feats_n, 1])            (L2, with appended ones column)
    pred_n = labelset[argmin_l ||f - c_l||]   with c = initc[labelset]  [128, 513]

Equivalent argmax formulation used on device (monotone transform, per row n):
    argmin_l dist  ==  argmax_l  T[n, l]
    T[n, l] = sum_d feats[n,d] * c[l,d]  +  cdv_l  +  u_n * w2neg_l
    cdv_l   = c[l,512] - MBAR * w2c_l      (per-l constant, exact f32)
    u_n     = sqrt(|feats_n|^2 + 1) - MBAR
    w2neg_l = -(0.5*|c_l|^2 - mean)        (centered, fp16)

Sharding: pure data-parallel over rows, N/8 = 32768 rows per NeuronCore.

Dataflow: host pre-packs feats in fp16 *transposed* layout
    ftpack[p, g, k*512 + n] = feats[g*512 + n, k*128 + p]
so the DMA streams matmul-ready fT tiles; the per-row norm term u is
host-computed and shipped as a [1, 32768] fp16 row. HBM traffic per
core: 32 MiB fp16.

Argmax via u32 score|index encoding: scores shifted positive (+4096 in
the ACT bias) so their f32 bit patterns order like the values as
unsigned ints; DVE clears the low 7 mantissa bits and ORs in (127 - l)
(both per-partition scalars in G-layout where partition == l). PE
transposes the encoded bits (pass-through for normal floats); a single
segmented tensor_reduce(max) per group then yields max-and-argmax for
all 4 subtiles in one DVE op; host decodes l = 127 - (v & 127). Ties
resolve to the smallest l, matching argmin's first-index rule.

Per-core dataflow (64 groups of 512 rows):
    DMA    : 1 MiB fT tile per 2 groups, alternating sync/gpsimd/scalar
             queues (first tile split 4x for an early compute start)
    PE     : 1x matmul  G[l,n] += w2neg16.T @ u_row   (K=1 rank-1, first
             so its LDWEIGHTS hides under the previous group's stream)
             4x matmul  G[l,n] += cT16_k.T @ fT_k     (fp16, N=512)
    ACT    : G_ps -> SBUF copy with bias = cdv + 4096  (exact f32)
    DVE    : encode S = (Gs & 0xFFFFFF80) | (127-l)    (tensor_scalar)
    PE     : 4x transpose 128x128 of encoded bits: S -> T_enc[n, l]
    DVE    : 1x tensor_reduce max [128,4,128] -> [128,4] per group

Notes on this walrus build:
  - one sync wait per engine instruction: residual multi-wait
    instructions are split via same-engine NoOps (_split_multiwait).
  - all matmuls in the G accumulation group are fp16 (mixing fp16 and
    fp32r in one PSUM group silently corrupts on hardware).
"""

import os
import sys

import numpy as np

for _p in ("/opt/trn_rl_repo",):
    if _p not in sys.path and os.path.isdir(_p):
        sys.path.insert(0, _p)

import concourse.bass as bass
import concourse.mybir as mybir
import concourse.tile as tile
from concourse.bass_utils import run_bass_kernel_spmd

N, D, K = 262144, 512, 128
N_CORES = 8
ROWS_PER_CORE = N // N_CORES  # 32768
GROUP = 512  # rows per compute group (4 subtiles of 128)
DMA_GROUPS = 2  # groups per DMA (1 MiB transfers)

F32 = mybir.dt.float32
F16 = mybir.dt.float16
U32 = mybir.dt.uint32
AF = mybir.ActivationFunctionType
ALU = mybir.AluOpType
AX = mybir.AxisListType

# fp16 constant pack [128, CPK16_W]
P16_CT = 0         # [128, 512] cT: cT[p, 128k+l] = c[l, 128k+p]
P16_W2 = 512       # row 0, [1, 128] w2neg = -(w2 - mean(w2))
CPK16_W = 640

# f32 constant pack [128, CPK32_W]
P32_IDENT = 0      # [128, 128] identity
P32_CDV = 128      # [128, 1] column: cdv_l + SHIFT
CPK32_W = 130

# u32 constant pack [128, 2]: col 0 = mantissa mask, col 1 = 127 - l
LP_MASK = 0
LP_LIDX = 1

MBAR = float(np.sqrt(513.0))  # fixed shift for m; argmax-invariant
SHIFT = 4096.0                # makes scores positive (u32-orderable bits)
ENC_MASK = 0xFFFFFF80         # clear low 7 mantissa bits for the l index


def _split_multiwait(nc):
    """Walrus (this build) allows one sync wait per engine instruction.

    Tile occasionally emits 2+ (data dep + buffer-slot release on another
    semaphore). Splitting is semantics-preserving: a same-engine NoOp placed
    immediately before the instruction carries the surplus waits; the engine
    executes in order, so all waits are still enforced before the instruction
    runs.
    """
    import bass_rust

    for fn in nc.m.functions:
        for blk in fn.blocks:
            out = []
            changed = False
            for inst in blk.instructions:
                si = getattr(inst, "sync_info", None)
                waits = list(si.on_wait) if si is not None else []
                if len(waits) > 1:
                    for w in waits[:-1]:
                        nop = mybir.InstNoOp(
                            name=nc.get_next_instruction_name(), ins=[], outs=[]
                        )
                        nop.engine = inst.engine
                        nop.sync_info = bass_rust.SyncInfo(
                            on_wait=[w], on_update=[]
                        )
                        out.append(nop)
                    inst.sync_info = bass_rust.SyncInfo(
                        on_wait=waits[-1:], on_update=list(si.on_update)
                    )
                    changed = True
                out.append(inst)
            if changed:
                blk.instructions = out


def build_core_program(rows=ROWS_PER_CORE, split_waits=True):
    """Bass program for one NeuronCore processing `rows` rows of feats."""
    assert rows % (GROUP * DMA_GROUPS) == 0
    ngroups = rows // GROUP           # 64
    ndma = ngroups // DMA_GROUPS      # 32

    nc = bass.Bass()
    ft_d = nc.declare_dram_parameter("ftpack", [128, ngroups, 2048], F16, isOutput=False)
    cpk16_d = nc.declare_dram_parameter("cpk16", [128, CPK16_W], F16, isOutput=False)
    cpk32_d = nc.declare_dram_parameter("cpk32", [128, CPK32_W], F32, isOutput=False)
    lpack_d = nc.declare_dram_parameter("lpack", [128, 2], U32, isOutput=False)
    u_d = nc.declare_dram_parameter("upack", [1, rows], F16, isOutput=False)
    idx_d = nc.declare_dram_parameter("idx8", [128, ngroups, 4], U32, isOutput=True)

    with tile.TileContext(nc) as tc:
        with (
            tc.tile_pool(name="const", bufs=1) as constp,
            tc.tile_pool(name="fin", bufs=3) as finp,
            tc.tile_pool(name="gs", bufs=2) as gsp,
            tc.tile_pool(name="enc", bufs=2) as encp,
            tc.tile_pool(name="small", bufs=1) as smallp,
            tc.tile_pool(name="outp", bufs=1) as outp,
            tc.tile_pool(name="gp", bufs=2, space="PSUM") as gpp,
            tc.tile_pool(name="tp", bufs=2, space="PSUM") as tpp,
        ):
            # constants spread across the three DMA queues so they land in
            # parallel right after each engine's preamble: the first matmuls
            # need cpk16 (scalar) + urow (gpsimd) + first ft piece (sync)
            cpk16 = constp.tile([128, CPK16_W], F16)
            nc.scalar.dma_start(cpk16[:], cpk16_d[:])
            urow = constp.tile([1, rows], F16)
            nc.gpsimd.dma_start(urow[:], u_d[:])
            lpack = constp.tile([128, 2], U32)
            nc.gpsimd.dma_start(lpack[:], lpack_d[:])
            cpk32 = constp.tile([128, CPK32_W], F32)
            nc.scalar.dma_start(cpk32[:], cpk32_d[:])

            cT16 = cpk16[:, P16_CT : P16_CT + 512]
            w2neg16_row = cpk16[0:1, P16_W2 : P16_W2 + 128]
            ident32 = cpk32[:, P32_IDENT : P32_IDENT + 128]
            cdv_col = cpk32[:, P32_CDV : P32_CDV + 1]
            mask_col = lpack[:, LP_MASK : LP_MASK + 1]
            lidx_col = lpack[:, LP_LIDX : LP_LIDX + 1]
            idxacc = outp.tile([128, ngroups, 4], U32)

            # warmup: make ACT/DVE observe their const-DMA lanes with a
            # single-wait instruction each; residual multi-waits elsewhere
            # are handled by _split_multiwait.
            act_warm = smallp.tile([1, 1], F32, tag="act_warm")
            nc.scalar.copy(act_warm[:], cpk32[0:1, 0:1])
            dve_warm = smallp.tile([1, 1], F16, tag="dve_warm")
            nc.vector.tensor_copy(dve_warm[:], urow[0:1, 0:1])

            def flush(p, S_p):
                # transpose encoded bits -> T[n, l]; the reduce must run in
                # f32: the u32 path mangles low mantissa bits (HW-verified),
                # f32 is bit-exact and orders the same for these positive
                # floats. One segmented tensor_reduce covers all 4 subtiles.
                T_ps = tpp.tile([128, 512], F32)
                Sf = S_p[:].bitcast(F32)
                for j in range(4):
                    nc.tensor.transpose(
                        T_ps[:, j * 128 : (j + 1) * 128],
                        Sf[:, j * 128 : (j + 1) * 128],
                        ident32,
                    )
                nc.vector.tensor_reduce(
                    idxacc[:, p, :].bitcast(F32),
                    T_ps[:].rearrange("q (a b) -> q a b", a=4),
                    axis=AX.X,
                    op=ALU.max,
                )
                if p % 16 == 15:
                    t0 = p - 15
                    nc.sync.dma_start(
                        idx_d[:, t0 : t0 + 16, :], idxacc[:, t0 : t0 + 16, :]
                    )

            # feature loads spread over three DMA queues (SWDGE + both
            # HWDGE). Each group's transposes are deferred until after the
            # NEXT group's matmuls (software pipelining): the in-order PE
            # would otherwise stall at group g's transposes waiting for the
            # DVE encode of g while g+1's independent matmuls sit queued.
            pending = None
            for dg in range(ndma):
                ft = finp.tile([128, DMA_GROUPS, 4, 512], F16)
                src = ft_d[:, dg * DMA_GROUPS : (dg + 1) * DMA_GROUPS, :]
                queue = (nc.sync, nc.gpsimd, nc.scalar)[dg % 3]
                if dg == 0:
                    # split the first tile so group 0's matmuls can start
                    # ~2.5us earlier (256 KiB first-chunk vs 1 MiB)
                    for h4 in range(DMA_GROUPS):
                        queue.dma_start(
                            ft[:, h4, 0:2, :], ft_d[:, h4, 0:1024]
                        )
                        queue.dma_start(
                            ft[:, h4, 2:4, :], ft_d[:, h4, 1024:2048]
                        )
                else:
                    queue.dma_start(ft[:], src)

                for h in range(DMA_GROUPS):
                    g = dg * DMA_GROUPS + h
                    G_ps = gpp.tile([128, 512], F32)
                    # rank-1 first: no ft dependency, and its LDWEIGHTS
                    # hides under the previous group's last stream
                    nc.tensor.matmul(
                        G_ps[:],
                        w2neg16_row,
                        urow[0:1, g * 512 : (g + 1) * 512],
                        start=True,
                        stop=False,
                    )
                    for k in range(4):
                        nc.tensor.matmul(
                            G_ps[:],
                            cT16[:, k * 128 : (k + 1) * 128],
                            ft[:, h, k, :],
                            start=False,
                            stop=(k == 3),
                        )
                    if pending is not None:
                        flush(*pending)
                    # PSUM -> SBUF with exact f32 per-l constant (+SHIFT)
                    Gs = gsp.tile([128, 512], F32)
                    nc.scalar.activation(
                        Gs[:], G_ps[:], AF.Identity, bias=cdv_col, scale=1.0
                    )
                    # encode (score | 127-l) in u32 bits; partition == l here
                    # (immediate mask: keeps the op eligible for 2x_2P mode)
                    S = encp.tile([128, 512], U32)
                    nc.vector.tensor_scalar(
                        S[:],
                        Gs[:].bitcast(U32),
                        int(ENC_MASK),
                        lidx_col,
                        op0=ALU.bitwise_and,
                        op1=ALU.bitwise_or,
                    )
                    pending = (g, S)

            flush(*pending)
    if split_waits:
        _split_multiwait(nc)
    return nc


def make_const_inputs(initc, labelset):
    c = np.asarray(initc, dtype=np.float32)[np.asarray(labelset).astype(np.int64)]
    assert c.shape == (K, D + 1)
    w2 = 0.5 * np.sum(c.astype(np.float64) ** 2, axis=1)
    # Centering w2 (and shifting m by MBAR) adds per-row constants to the
    # scores (argmax invariant) while keeping the fp16-rounded rank-1 term
    # u*w2c small. The per-l constant cdv ships exact as an f32 ACT bias.
    w2c = w2 - w2.mean()
    cdv = (c[:, D].astype(np.float64) - MBAR * w2c + SHIFT).astype(np.float32)

    cpk16 = np.zeros((128, CPK16_W), np.float16)
    for k in range(4):
        cpk16[:, P16_CT + k * 128 : P16_CT + (k + 1) * 128] = (
            c[:, k * 128 : (k + 1) * 128].T.astype(np.float16)
        )
    cpk16[0, P16_W2 : P16_W2 + 128] = (-w2c).astype(np.float16)

    cpk32 = np.zeros((128, CPK32_W), np.float32)
    cpk32[:, P32_IDENT : P32_IDENT + 128] = np.eye(128, dtype=np.float32)
    cpk32[:, P32_CDV] = cdv

    lpack = np.zeros((128, 2), np.uint32)
    lpack[:, LP_MASK] = np.uint32(ENC_MASK)
    lpack[:, LP_LIDX] = (127 - np.arange(128)).astype(np.uint32)
    return {"cpk16": cpk16, "cpk32": cpk32, "lpack": lpack}


def pack_feats_core(shard16):
    """[32768, 512] fp16 -> [128, 64, 2048]: P[p, g, k*512+n] = s[g*512+n, k*128+p]."""
    ngroups = shard16.shape[0] // GROUP
    P = shard16.reshape(ngroups, GROUP, 4, 128).transpose(3, 0, 2, 1)
    return np.ascontiguousarray(P).reshape(128, ngroups, 2048)


def build_in_maps(feats, initc, labelset):
    feats = np.asarray(feats, dtype=np.float32)
    consts = make_const_inputs(initc, labelset)

    feats16 = feats.astype(np.float16)
    q = np.einsum("nd,nd->n", feats, feats, dtype=np.float64)
    u16 = (np.sqrt(q + 1.0) - MBAR).astype(np.float16)

    in_maps = []
    for core in range(N_CORES):
        sl = slice(core * ROWS_PER_CORE, (core + 1) * ROWS_PER_CORE)
        in_maps.append(
            {
                "ftpack": pack_feats_core(feats16[sl]),
                "upack": np.ascontiguousarray(u16[sl].reshape(1, -1)),
                **consts,
            }
        )
    return in_maps


def kernel(feats, initc, labelset):
    labelset_np = np.asarray(labelset)
    in_maps = build_in_maps(feats, initc, labelset)
    nc = build_core_program(ROWS_PER_CORE)
    res = run_bass_kernel_spmd(nc, in_maps, list(range(N_CORES)))

    preds = []
    for core in range(N_CORES):
        enc = np.asarray(res.results[core]["idx8"])  # [128, ngroups, 4] u32
        idx = (127 - (enc & np.uint32(127))).astype(np.int64)
        # [p, g, j] -> row 512*g + 128*j + p
        preds.append(idx.transpose(1, 2, 0).reshape(-1))
    idx_all = np.concatenate(preds)
    return labelset_np[idx_all]


# revision 38
# speedup vs baseline: 1.1135x; 1.1135x over previous
"""Trainium2 Bass kernel: nearest-centroid assignment (vq_codebook).

Computes, for each row of `feats` [N, 512]:
    f = normalize([feats_n, 1])            (L2, with appended ones column)
    pred_n = labelset[argmin_l ||f - c_l||]   with c = initc[labelset]  [128, 513]

Equivalent argmax formulation used on device (monotone transform, per row n):
    argmin_l dist  ==  argmax_l  T[n, l]
    T[n, l] = sum_d feats[n,d] * c[l,d]  +  cdv_l  +  u_n * w2neg_l
    cdv_l   = c[l,512] - MBAR * w2c_l      (per-l constant, exact f32)
    u_n     = sqrt(|feats_n|^2 + 1) - MBAR
    w2neg_l = -(0.5*|c_l|^2 - mean)        (centered, fp16)

Sharding: pure data-parallel over rows, N/8 = 32768 rows per NeuronCore.

Dataflow: host pre-packs feats in fp16 *transposed* layout
    ftpack[p, g, k*512 + n] = feats[g*512 + n, k*128 + p]
so the DMA streams matmul-ready fT tiles; the per-row norm term u is
host-computed and shipped as a [1, 32768] fp16 row. HBM traffic per
core: 32 MiB fp16.

Argmax via u32 score|index encoding: scores shifted positive (+4096 in
the ACT bias) so their f32 bit patterns order like the values as
unsigned ints; DVE clears the low 7 mantissa bits and ORs in (127 - l)
(both per-partition scalars in G-layout where partition == l). PE
transposes the encoded bits (pass-through for normal floats); a single
segmented tensor_reduce(max) per group then yields max-and-argmax for
all 4 subtiles in one DVE op; host decodes l = 127 - (v & 127). Ties
resolve to the smallest l, matching argmin's first-index rule.

Per-core dataflow (64 groups of 512 rows):
    DMA    : 1 MiB fT tile per 2 groups over sync/gpsimd/scalar queues,
             issued one tile ahead with 6 tiles of SBUF lookahead (the
             DMA never waits on compute); groups 0-3 stream as 256 KiB
             pieces on the sync ring for a fast ramp
    PE     : rank-1 matmuls G[l,n] += w2neg16.T @ u_row batched 4
             groups at a time (the K=1 op forces a PE array-config
             switch vs its K=128 neighbors -- pay it once per 4 groups),
             then per group 4x matmul G[l,n] += cT16_k.T @ fT_k (fp16,
             N=512) with the transposes of group g-2 interleaved
             between the streams (their f32 LDWEIGHTS hide fully)
    ACT    : G_ps -> SBUF copy with bias = cdv + 4096  (exact f32)
    DVE    : encode S = (Gs & 0xFFFFFF80) | (127-l)    (tensor_scalar)
    PE     : 4x transpose 128x128 of encoded bits: S -> T_enc[n, l]
    DVE    : 1x segmented tensor_reduce max [128,4,128] -> [128,4]
Steady state is ~1570ns per 512-row group on the PE, within ~5% of the
PE stream floor; end-to-end is co-limited by PE and the 32 MiB/core
HBM stream (~358 GB/s per-NC cap).

Notes on this walrus build:
  - one sync wait per engine instruction: residual multi-wait
    instructions are split via same-engine NoOps (_split_multiwait).
  - all matmuls in the G accumulation group are fp16 (mixing fp16 and
    fp32r in one PSUM group silently corrupts on hardware).
"""

import os
import sys

import numpy as np

for _p in ("/opt/trn_rl_repo",):
    if _p not in sys.path and os.path.isdir(_p):
        sys.path.insert(0, _p)

import concourse.bass as bass
import concourse.mybir as mybir
import concourse.tile as tile
from concourse.bass_utils import run_bass_kernel_spmd

N, D, K = 262144, 512, 128
N_CORES = 8
ROWS_PER_CORE = N // N_CORES  # 32768
GROUP = 512  # rows per compute group (4 subtiles of 128)
DMA_GROUPS = 2  # groups per DMA (1 MiB transfers)

F32 = mybir.dt.float32
F16 = mybir.dt.float16
U32 = mybir.dt.uint32
AF = mybir.ActivationFunctionType
ALU = mybir.AluOpType
AX = mybir.AxisListType

# fp16 constant pack [128, CPK16_W]
P16_CT = 0         # [128, 512] cT: cT[p, 128k+l] = c[l, 128k+p]
P16_W2 = 512       # row 0, [1, 128] w2neg = -(w2 - mean(w2))
CPK16_W = 640

# f32 constant pack [128, CPK32_W]
P32_IDENT = 0      # [128, 128] identity
P32_CDV = 128      # [128, 1] column: cdv_l + SHIFT
CPK32_W = 130

# u32 constant pack [128, 2]: col 0 = mantissa mask, col 1 = 127 - l
LP_MASK = 0
LP_LIDX = 1

MBAR = float(np.sqrt(513.0))  # fixed shift for m; argmax-invariant
SHIFT = 4096.0                # makes scores positive (u32-orderable bits)
ENC_MASK = 0xFFFFFF80         # clear low 7 mantissa bits for the l index


def _split_multiwait(nc):
    """Walrus (this build) allows one sync wait per engine instruction.

    Tile occasionally emits 2+ (data dep + buffer-slot release on another
    semaphore). Splitting is semantics-preserving: a same-engine NoOp placed
    immediately before the instruction carries the surplus waits; the engine
    executes in order, so all waits are still enforced before the instruction
    runs.
    """
    import bass_rust

    for fn in nc.m.functions:
        for blk in fn.blocks:
            out = []
            changed = False
            for inst in blk.instructions:
                si = getattr(inst, "sync_info", None)
                waits = list(si.on_wait) if si is not None else []
                if len(waits) > 1:
                    for w in waits[:-1]:
                        nop = mybir.InstNoOp(
                            name=nc.get_next_instruction_name(), ins=[], outs=[]
                        )
                        nop.engine = inst.engine
                        nop.sync_info = bass_rust.SyncInfo(
                            on_wait=[w], on_update=[]
                        )
                        out.append(nop)
                    inst.sync_info = bass_rust.SyncInfo(
                        on_wait=waits[-1:], on_update=list(si.on_update)
                    )
                    changed = True
                out.append(inst)
            if changed:
                blk.instructions = out


def build_core_program(rows=ROWS_PER_CORE, split_waits=True):
    """Bass program for one NeuronCore processing `rows` rows of feats."""
    assert rows % (GROUP * DMA_GROUPS) == 0
    ngroups = rows // GROUP           # 64
    ndma = ngroups // DMA_GROUPS      # 32

    nc = bass.Bass()
    ft_d = nc.declare_dram_parameter("ftpack", [128, ngroups, 2048], F16, isOutput=False)
    cpk16_d = nc.declare_dram_parameter("cpk16", [128, CPK16_W], F16, isOutput=False)
    cpk32_d = nc.declare_dram_parameter("cpk32", [128, CPK32_W], F32, isOutput=False)
    lpack_d = nc.declare_dram_parameter("lpack", [128, 2], U32, isOutput=False)
    u_d = nc.declare_dram_parameter("upack", [1, rows], F16, isOutput=False)
    idx_d = nc.declare_dram_parameter("idx8", [128, ngroups, 4], U32, isOutput=True)

    with tile.TileContext(nc) as tc:
        with (
            tc.tile_pool(name="const", bufs=1) as constp,
            tc.tile_pool(name="fin", bufs=6) as finp,
            tc.tile_pool(name="gs", bufs=2) as gsp,
            tc.tile_pool(name="enc", bufs=3) as encp,
            tc.tile_pool(name="small", bufs=1) as smallp,
            tc.tile_pool(name="outp", bufs=1) as outp,
            tc.tile_pool(name="gp", bufs=5, space="PSUM") as gpp,
            tc.tile_pool(name="tp", bufs=2, space="PSUM") as tpp,
        ):
            # constants spread across the three DMA queues so they land in
            # parallel right after each engine's preamble: the first matmuls
            # need cpk16 (scalar) + urow (gpsimd) + first ft piece (sync)
            cpk16 = constp.tile([128, CPK16_W], F16)
            nc.scalar.dma_start(cpk16[:], cpk16_d[:])
            urow = constp.tile([1, rows], F16)
            nc.sync.dma_start(urow[:], u_d[:])
            lpack = constp.tile([128, 2], U32)
            nc.sync.dma_start(lpack[:], lpack_d[:])
            cpk32 = constp.tile([128, CPK32_W], F32)
            nc.scalar.dma_start(cpk32[:], cpk32_d[:])

            cT16 = cpk16[:, P16_CT : P16_CT + 512]
            w2neg16_row = cpk16[0:1, P16_W2 : P16_W2 + 128]
            ident32 = cpk32[:, P32_IDENT : P32_IDENT + 128]
            cdv_col = cpk32[:, P32_CDV : P32_CDV + 1]
            mask_col = lpack[:, LP_MASK : LP_MASK + 1]
            lidx_col = lpack[:, LP_LIDX : LP_LIDX + 1]
            idxacc = outp.tile([128, ngroups, 4], U32)

            # warmup: make ACT/DVE observe their const-DMA lanes with a
            # single-wait instruction each; residual multi-waits elsewhere
            # are handled by _split_multiwait.
            act_warm = smallp.tile([1, 1], F32, tag="act_warm")
            nc.scalar.copy(act_warm[:], cpk32[0:1, 0:1])
            dve_warm = smallp.tile([1, 1], F16, tag="dve_warm")
            nc.vector.tensor_copy(dve_warm[:], urow[0:1, 0:1])

            def finish(p, T_ps):
                # one segmented tensor_reduce covers all 4 subtiles; must
                # run in f32: the u32 path mangles low mantissa bits
                # (HW-verified), f32 is bit-exact and orders the same for
                # these positive floats.
                nc.vector.tensor_reduce(
                    idxacc[:, p, :].bitcast(F32),
                    T_ps[:].rearrange("q (a b) -> q a b", a=4),
                    axis=AX.X,
                    op=ALU.max,
                )
                # flush results; the final group ships alone so the very
                # last DMA (on the critical tail) is tiny
                sched = {15: (0, 16), 31: (16, 16), 47: (32, 16),
                         62: (48, 15), 63: (63, 1)}
                if p in sched:
                    t0, cnt = sched[p]
                    nc.sync.dma_start(
                        idx_d[:, t0 : t0 + cnt, :], idxacc[:, t0 : t0 + cnt, :]
                    )

            # feature loads spread over three DMA queues (SWDGE + both
            # HWDGE). Each group's transposes are deferred TWO groups and
            # interleaved between that later group's matmul streams
            # (software pipelining): the encode of group g completes ~1.5us
            # after g's last matmul, so transposes interleaved into g+1
            # would stall the in-order PE; into g+2 they never wait.
            pend1 = None  # (g-1, S)
            pend2 = None  # (g-2, S) -- transposed during this group
            tiles = {}
            G_banks = {}

            def issue_dma(dg):
                # ramp: groups 0-3 stream as 256 KiB pieces on the sync
                # HWDGE ring (fastest spin-up; SWDGE takes ~3us longer to
                # first byte). Steady state rotates all three queues, with
                # gpsimd taking the first big tile while sync drains pieces.
                ft = finp.tile([128, DMA_GROUPS, 4, 512], F16)
                src = ft_d[:, dg * DMA_GROUPS : (dg + 1) * DMA_GROUPS, :]
                if dg < 2:
                    for h4 in range(DMA_GROUPS):
                        gq = dg * DMA_GROUPS + h4
                        nc.sync.dma_start(ft[:, h4, 0:2, :], ft_d[:, gq, 0:1024])
                        nc.sync.dma_start(ft[:, h4, 2:4, :], ft_d[:, gq, 1024:2048])
                elif dg == 2:
                    # scalar joins right after its two const loads
                    nc.scalar.dma_start(ft[:], src)
                else:
                    queue = (nc.gpsimd, nc.sync, nc.scalar)[dg % 3]
                    queue.dma_start(ft[:], src)
                tiles[dg] = ft

            issue_dma(0)
            for dg in range(ndma):
                # prefetch one tile ahead so the scalar-queue DMAs issue
                # before that engine gets busy with the next ACTIVATEs
                if dg + 1 < ndma:
                    issue_dma(dg + 1)
                ft = tiles.pop(dg)

                for h in range(DMA_GROUPS):
                    g = dg * DMA_GROUPS + h
                    # rank-1s batched 4 groups at a time: the K=1 matmul
                    # forces a PE array-config switch (~100ns each way)
                    # against the K=128 neighbors, so pay it once per 4
                    # groups instead of once per group. No ft dependency.
                    if g % 4 == 0:
                        for q in range(g, g + 4):
                            G_banks[q] = gpp.tile([128, 512], F32, name="G_ps")
                            nc.tensor.matmul(
                                G_banks[q][:],
                                w2neg16_row,
                                urow[0:1, q * 512 : (q + 1) * 512],
                                start=True,
                                stop=False,
                            )
                    G_ps = G_banks.pop(g)
                    if pend2 is not None:
                        p_prev, S_prev = pend2
                        T_ps = tpp.tile([128, 512], F32)
                        Sf = S_prev[:].bitcast(F32)
                    for k in range(4):
                        nc.tensor.matmul(
                            G_ps[:],
                            cT16[:, k * 128 : (k + 1) * 128],
                            ft[:, h, k, :],
                            start=False,
                            stop=(k == 3),
                        )
                        if pend2 is not None:
                            nc.tensor.transpose(
                                T_ps[:, k * 128 : (k + 1) * 128],
                                Sf[:, k * 128 : (k + 1) * 128],
                                ident32,
                            )
                    if pend2 is not None:
                        finish(p_prev, T_ps)
                    # PSUM -> SBUF with exact f32 per-l constant (+SHIFT)
                    Gs = gsp.tile([128, 512], F32)
                    nc.scalar.activation(
                        Gs[:], G_ps[:], AF.Identity, bias=cdv_col, scale=1.0
                    )
                    # encode (score | 127-l) in u32 bits; partition == l here
                    # (immediate mask: keeps the op eligible for 2x_2P mode)
                    S = encp.tile([128, 512], U32)
                    nc.vector.tensor_scalar(
                        S[:],
                        Gs[:].bitcast(U32),
                        int(ENC_MASK),
                        lidx_col,
                        op0=ALU.bitwise_and,
                        op1=ALU.bitwise_or,
                    )
                    pend2, pend1 = pend1, (g, S)

            # tail flush for the final two groups
            for p_prev, S_prev in (pend2, pend1):
                T_ps = tpp.tile([128, 512], F32)
                Sf = S_prev[:].bitcast(F32)
                for j in range(4):
                    nc.tensor.transpose(
                        T_ps[:, j * 128 : (j + 1) * 128],
                        Sf[:, j * 128 : (j + 1) * 128],
                        ident32,
                    )
                finish(p_prev, T_ps)
    if split_waits:
        _split_multiwait(nc)
    return nc


def make_const_inputs(initc, labelset):
    c = np.asarray(initc, dtype=np.float32)[np.asarray(labelset).astype(np.int64)]
    assert c.shape == (K, D + 1)
    w2 = 0.5 * np.sum(c.astype(np.float64) ** 2, axis=1)
    # Centering w2 (and shifting m by MBAR) adds per-row constants to the
    # scores (argmax invariant) while keeping the fp16-rounded rank-1 term
    # u*w2c small. The per-l constant cdv ships exact as an f32 ACT bias.
    w2c = w2 - w2.mean()
    cdv = (c[:, D].astype(np.float64) - MBAR * w2c + SHIFT).astype(np.float32)

    cpk16 = np.zeros((128, CPK16_W), np.float16)
    for k in range(4):
        cpk16[:, P16_CT + k * 128 : P16_CT + (k + 1) * 128] = (
            c[:, k * 128 : (k + 1) * 128].T.astype(np.float16)
        )
    cpk16[0, P16_W2 : P16_W2 + 128] = (-w2c).astype(np.float16)

    cpk32 = np.zeros((128, CPK32_W), np.float32)
    cpk32[:, P32_IDENT : P32_IDENT + 128] = np.eye(128, dtype=np.float32)
    cpk32[:, P32_CDV] = cdv

    lpack = np.zeros((128, 2), np.uint32)
    lpack[:, LP_MASK] = np.uint32(ENC_MASK)
    lpack[:, LP_LIDX] = (127 - np.arange(128)).astype(np.uint32)
    return {"cpk16": cpk16, "cpk32": cpk32, "lpack": lpack}


def pack_feats_core(shard16):
    """[32768, 512] fp16 -> [128, 64, 2048]: P[p, g, k*512+n] = s[g*512+n, k*128+p]."""
    ngroups = shard16.shape[0] // GROUP
    P = shard16.reshape(ngroups, GROUP, 4, 128).transpose(3, 0, 2, 1)
    return np.ascontiguousarray(P).reshape(128, ngroups, 2048)


def build_in_maps(feats, initc, labelset):
    feats = np.asarray(feats, dtype=np.float32)
    consts = make_const_inputs(initc, labelset)

    feats16 = feats.astype(np.float16)
    q = np.einsum("nd,nd->n", feats, feats, dtype=np.float64)
    u16 = (np.sqrt(q + 1.0) - MBAR).astype(np.float16)

    in_maps = []
    for core in range(N_CORES):
        sl = slice(core * ROWS_PER_CORE, (core + 1) * ROWS_PER_CORE)
        in_maps.append(
            {
                "ftpack": pack_feats_core(feats16[sl]),
                "upack": np.ascontiguousarray(u16[sl].reshape(1, -1)),
                **consts,
            }
        )
    return in_maps


def kernel(feats, initc, labelset):
    labelset_np = np.asarray(labelset)
    in_maps = build_in_maps(feats, initc, labelset)
    nc = build_core_program(ROWS_PER_CORE)
    res = run_bass_kernel_spmd(nc, in_maps, list(range(N_CORES)))

    preds = []
    for core in range(N_CORES):
        enc = np.asarray(res.results[core]["idx8"])  # [128, ngroups, 4] u32
        idx = (127 - (enc & np.uint32(127))).astype(np.int64)
        # [p, g, j] -> row 512*g + 128*j + p
        preds.append(idx.transpose(1, 2, 0).reshape(-1))
    idx_all = np.concatenate(preds)
    return labelset_np[idx_all]


# revision 51
# speedup vs baseline: 1.1141x; 1.0006x over previous
"""Trainium2 Bass kernel: nearest-centroid assignment (vq_codebook).

Computes, for each row of `feats` [N, 512]:
    f = normalize([feats_n, 1])            (L2, with appended ones column)
    pred_n = labelset[argmin_l ||f - c_l||]   with c = initc[labelset]  [128, 513]

Equivalent argmax formulation used on device (monotone transform, per row n):
    argmin_l dist  ==  argmax_l  T[n, l]
    T[n, l] = sum_d feats[n,d] * c[l,d]  +  cdv_l  +  u_n * w2neg_l
    cdv_l   = c[l,512] - MBAR * w2c_l      (per-l constant, exact f32)
    u_n     = sqrt(|feats_n|^2 + 1) - MBAR
    w2neg_l = -(0.5*|c_l|^2 - mean)        (centered, fp16)

Sharding: pure data-parallel over rows, N/8 = 32768 rows per NeuronCore.

Dataflow: host pre-packs feats in fp16 *transposed* layout
    ftpack[p, g, k*512 + n] = feats[g*512 + n, k*128 + p]
so the DMA streams matmul-ready fT tiles; the per-row norm term u is
host-computed and shipped as a [1, 32768] fp16 row. HBM traffic per
core: 32 MiB fp16.

Argmax via u32 score|index encoding: scores shifted positive (+4096 in
the ACT bias) so their f32 bit patterns order like the values as
unsigned ints; DVE clears the low 7 mantissa bits and ORs in (127 - l)
(both per-partition scalars in G-layout where partition == l). PE
transposes the encoded bits (pass-through for normal floats); a single
segmented tensor_reduce(max) per group then yields max-and-argmax for
all 4 subtiles in one DVE op; host decodes l = 127 - (v & 127). Ties
resolve to the smallest l, matching argmin's first-index rule.

Per-core dataflow (64 groups of 512 rows):
    DMA    : 1 MiB fT tile per 2 groups over sync/gpsimd/scalar queues,
             issued one tile ahead with 6 tiles of SBUF lookahead (so a
             PE stall never idles the DMA rings); groups 0-3 stream as
             256 KiB pieces on the sync ring for a fast ramp
    PE     : rank-1 matmuls G[l,n] += w2neg16.T @ u_row batched 4
             groups at a time (the K=1 op forces a PE array-config
             switch vs its K=128 neighbors -- pay it once per 4 groups)
             4x matmul  G[l,n] += cT16_k.T @ fT_k     (fp16, N=512)
             with group g-2's transposes interleaved between the
             streams (their f32 LDWEIGHTS hide fully; 2-group deferral
             because the encode lands ~1.5us after a group's last MM)
    ACT    : G_ps -> SBUF copy with bias = cdv + 4096  (exact f32)
    DVE    : encode S = (Gs & 0xFFFFFF80) | (127-l)    (tensor_scalar)
    PE     : 4x transpose 128x128 of encoded bits: S -> T_enc[n, l]
    DVE    : 1x segmented tensor_reduce max [128,4,128] -> [128,4]
Steady state: ~1570ns per 512-row group on PE (~5% over the stream
floor); end-to-end co-limited by PE and the 32 MiB/core HBM read.

Notes on this walrus build:
  - one sync wait per engine instruction: residual multi-wait
    instructions are split via same-engine NoOps (_split_multiwait).
  - all matmuls in the G accumulation group are fp16 (mixing fp16 and
    fp32r in one PSUM group silently corrupts on hardware).
"""

import os
import sys

import numpy as np

for _p in ("/opt/trn_rl_repo",):
    if _p not in sys.path and os.path.isdir(_p):
        sys.path.insert(0, _p)

import concourse.bass as bass
import concourse.mybir as mybir
import concourse.tile as tile
from concourse.bass_utils import run_bass_kernel_spmd

N, D, K = 262144, 512, 128
N_CORES = 8
ROWS_PER_CORE = N // N_CORES  # 32768
GROUP = 512  # rows per compute group (4 subtiles of 128)
DMA_GROUPS = 2  # groups per DMA (1 MiB transfers)

F32 = mybir.dt.float32
F16 = mybir.dt.float16
U32 = mybir.dt.uint32
AF = mybir.ActivationFunctionType
ALU = mybir.AluOpType
AX = mybir.AxisListType

# fp16 constant pack [128, CPK16_W]
P16_CT = 0         # [128, 512] cT: cT[p, 128k+l] = c[l, 128k+p]
P16_W2 = 512       # row 0, [1, 128] w2neg = -(w2 - mean(w2))
CPK16_W = 640

# f32 constant pack [128, CPK32_W]
P32_IDENT = 0      # [128, 128] identity
P32_CDV = 128      # [128, 1] column: cdv_l + SHIFT
CPK32_W = 130

# u32 constant pack [128, 2]: col 0 = mantissa mask, col 1 = 127 - l
LP_MASK = 0
LP_LIDX = 1

MBAR = float(np.sqrt(513.0))  # fixed shift for m; argmax-invariant
SHIFT = 4096.0                # makes scores positive (u32-orderable bits)
ENC_MASK = 0xFFFFFF80         # clear low 7 mantissa bits for the l index


def _split_multiwait(nc):
    """Walrus (this build) allows one sync wait per engine instruction.

    Tile occasionally emits 2+ (data dep + buffer-slot release on another
    semaphore). Splitting is semantics-preserving: a same-engine NoOp placed
    immediately before the instruction carries the surplus waits; the engine
    executes in order, so all waits are still enforced before the instruction
    runs.
    """
    import bass_rust

    for fn in nc.m.functions:
        for blk in fn.blocks:
            out = []
            changed = False
            for inst in blk.instructions:
                si = getattr(inst, "sync_info", None)
                waits = list(si.on_wait) if si is not None else []
                if len(waits) > 1:
                    for w in waits[:-1]:
                        nop = mybir.InstNoOp(
                            name=nc.get_next_instruction_name(), ins=[], outs=[]
                        )
                        nop.engine = inst.engine
                        nop.sync_info = bass_rust.SyncInfo(
                            on_wait=[w], on_update=[]
                        )
                        out.append(nop)
                    inst.sync_info = bass_rust.SyncInfo(
                        on_wait=waits[-1:], on_update=list(si.on_update)
                    )
                    changed = True
                out.append(inst)
            if changed:
                blk.instructions = out


def build_core_program(rows=ROWS_PER_CORE, split_waits=True):
    """Bass program for one NeuronCore processing `rows` rows of feats."""
    assert rows % (GROUP * DMA_GROUPS) == 0
    ngroups = rows // GROUP           # 64
    ndma = ngroups // DMA_GROUPS      # 32

    nc = bass.Bass()
    ft_d = nc.declare_dram_parameter("ftpack", [128, ngroups, 2048], F16, isOutput=False)
    cpk16_d = nc.declare_dram_parameter("cpk16", [128, CPK16_W], F16, isOutput=False)
    cpk32_d = nc.declare_dram_parameter("cpk32", [128, CPK32_W], F32, isOutput=False)
    lpack_d = nc.declare_dram_parameter("lpack", [128, 2], U32, isOutput=False)
    u_d = nc.declare_dram_parameter("upack", [1, rows], F16, isOutput=False)
    idx_d = nc.declare_dram_parameter("idx8", [128, ngroups, 4], U32, isOutput=True)

    with tile.TileContext(nc) as tc:
        with (
            tc.tile_pool(name="const", bufs=1) as constp,
            tc.tile_pool(name="fin", bufs=6) as finp,
            tc.tile_pool(name="gs", bufs=2) as gsp,
            tc.tile_pool(name="enc", bufs=3) as encp,
            tc.tile_pool(name="small", bufs=1) as smallp,
            tc.tile_pool(name="outp", bufs=1) as outp,
            tc.tile_pool(name="gp", bufs=5, space="PSUM") as gpp,
            tc.tile_pool(name="tp", bufs=2, space="PSUM") as tpp,
        ):
            # constants spread across the three DMA queues so they land in
            # parallel right after each engine's preamble: the first matmuls
            # need cpk16 (scalar) + urow (gpsimd) + first ft piece (sync)
            cpk16 = constp.tile([128, CPK16_W], F16)
            nc.scalar.dma_start(cpk16[:], cpk16_d[:])
            urow = constp.tile([1, rows], F16)
            nc.sync.dma_start(urow[:], u_d[:])
            lpack = constp.tile([128, 2], U32)
            nc.sync.dma_start(lpack[:], lpack_d[:])
            cpk32 = constp.tile([128, CPK32_W], F32)
            nc.scalar.dma_start(cpk32[:], cpk32_d[:])

            cT16 = cpk16[:, P16_CT : P16_CT + 512]
            w2neg16_row = cpk16[0:1, P16_W2 : P16_W2 + 128]
            ident32 = cpk32[:, P32_IDENT : P32_IDENT + 128]
            cdv_col = cpk32[:, P32_CDV : P32_CDV + 1]
            mask_col = lpack[:, LP_MASK : LP_MASK + 1]
            lidx_col = lpack[:, LP_LIDX : LP_LIDX + 1]
            idxacc = outp.tile([128, ngroups, 4], U32)

            # warmup: make ACT/DVE observe their const-DMA lanes with a
            # single-wait instruction each; residual multi-waits elsewhere
            # are handled by _split_multiwait.
            act_warm = smallp.tile([1, 1], F32, tag="act_warm")
            nc.scalar.copy(act_warm[:], cpk32[0:1, 0:1])
            dve_warm = smallp.tile([1, 1], F16, tag="dve_warm")
            nc.vector.tensor_copy(dve_warm[:], urow[0:1, 0:1])

            def finish(p, T_ps):
                # one segmented tensor_reduce covers all 4 subtiles; must
                # run in f32: the u32 path mangles low mantissa bits
                # (HW-verified), f32 is bit-exact and orders the same for
                # these positive floats.
                nc.vector.tensor_reduce(
                    idxacc[:, p, :].bitcast(F32),
                    T_ps[:].rearrange("q (a b) -> q a b", a=4),
                    axis=AX.X,
                    op=ALU.max,
                )
                # flush results; the final group ships alone so the very
                # last DMA (on the critical tail) is tiny
                sched = {15: (0, 16), 31: (16, 16), 47: (32, 16),
                         62: (48, 15), 63: (63, 1)}
                if p in sched:
                    t0, cnt = sched[p]
                    nc.sync.dma_start(
                        idx_d[:, t0 : t0 + cnt, :], idxacc[:, t0 : t0 + cnt, :]
                    )

            # feature loads spread over three DMA queues (SWDGE + both
            # HWDGE). Each group's transposes are deferred TWO groups and
            # interleaved between that later group's matmul streams
            # (software pipelining): the encode of group g completes ~1.5us
            # after g's last matmul, so transposes interleaved into g+1
            # would stall the in-order PE; into g+2 they never wait.
            pend1 = None  # (g-1, S)
            pend2 = None  # (g-2, S) -- transposed during this group
            tiles = {}
            G_banks = {}

            def issue_dma(dg):
                # ramp: groups 0-3 stream as 256 KiB pieces on the sync
                # HWDGE ring (fastest spin-up; SWDGE takes ~3us longer to
                # first byte). Steady state rotates all three queues, with
                # gpsimd taking the first big tile while sync drains pieces.
                ft = finp.tile([128, DMA_GROUPS, 4, 512], F16)
                src = ft_d[:, dg * DMA_GROUPS : (dg + 1) * DMA_GROUPS, :]
                if dg < 2:
                    for h4 in range(DMA_GROUPS):
                        gq = dg * DMA_GROUPS + h4
                        nc.sync.dma_start(ft[:, h4, 0:2, :], ft_d[:, gq, 0:1024])
                        nc.sync.dma_start(ft[:, h4, 2:4, :], ft_d[:, gq, 1024:2048])
                elif dg == 2:
                    # scalar joins right after its two const loads
                    nc.scalar.dma_start(ft[:], src)
                else:
                    queue = (nc.gpsimd, nc.sync, nc.scalar)[dg % 3]
                    queue.dma_start(ft[:], src)
                tiles[dg] = ft

            issue_dma(0)
            for dg in range(ndma):
                # prefetch one tile ahead so the scalar-queue DMAs issue
                # before that engine gets busy with the next ACTIVATEs
                if dg + 1 < ndma:
                    issue_dma(dg + 1)
                ft = tiles.pop(dg)

                for h in range(DMA_GROUPS):
                    g = dg * DMA_GROUPS + h
                    # rank-1s batched 4 groups at a time: the K=1 matmul
                    # forces a PE array-config switch (~100ns each way)
                    # against the K=128 neighbors, so pay it once per 4
                    # groups instead of once per group. No ft dependency.
                    if g % 4 == 0:
                        for q in range(g, g + 4):
                            G_banks[q] = gpp.tile([128, 512], F32, name="G_ps")
                            nc.tensor.matmul(
                                G_banks[q][:],
                                w2neg16_row,
                                urow[0:1, q * 512 : (q + 1) * 512],
                                start=True,
                                stop=False,
                            )
                    G_ps = G_banks.pop(g)
                    if pend2 is not None:
                        p_prev, S_prev = pend2
                        T_ps = tpp.tile([128, 512], F32)
                        Sf = S_prev[:].bitcast(F32)
                    for k in range(4):
                        nc.tensor.matmul(
                            G_ps[:],
                            cT16[:, k * 128 : (k + 1) * 128],
                            ft[:, h, k, :],
                            start=False,
                            stop=(k == 3),
                        )
                        if pend2 is not None:
                            nc.tensor.transpose(
                                T_ps[:, k * 128 : (k + 1) * 128],
                                Sf[:, k * 128 : (k + 1) * 128],
                                ident32,
                            )
                    if pend2 is not None:
                        finish(p_prev, T_ps)
                    # PSUM -> SBUF with exact f32 per-l constant (+SHIFT)
                    Gs = gsp.tile([128, 512], F32)
                    nc.scalar.activation(
                        Gs[:], G_ps[:], AF.Identity, bias=cdv_col, scale=1.0
                    )
                    # encode (score | 127-l) in u32 bits; partition == l here
                    # (immediate mask: keeps the op eligible for 2x_2P mode)
                    S = encp.tile([128, 512], U32)
                    nc.vector.tensor_scalar(
                        S[:],
                        Gs[:].bitcast(U32),
                        int(ENC_MASK),
                        lidx_col,
                        op0=ALU.bitwise_and,
                        op1=ALU.bitwise_or,
                    )
                    pend2, pend1 = pend1, (g, S)

            # tail flush for the final two groups
            for p_prev, S_prev in (pend2, pend1):
                T_ps = tpp.tile([128, 512], F32)
                Sf = S_prev[:].bitcast(F32)
                for j in range(4):
                    nc.tensor.transpose(
                        T_ps[:, j * 128 : (j + 1) * 128],
                        Sf[:, j * 128 : (j + 1) * 128],
                        ident32,
                    )
                finish(p_prev, T_ps)
    if split_waits:
        _split_multiwait(nc)
    return nc


def make_const_inputs(initc, labelset):
    c = np.asarray(initc, dtype=np.float32)[np.asarray(labelset).astype(np.int64)]
    assert c.shape == (K, D + 1)
    w2 = 0.5 * np.sum(c.astype(np.float64) ** 2, axis=1)
    # Centering w2 (and shifting m by MBAR) adds per-row constants to the
    # scores (argmax invariant) while keeping the fp16-rounded rank-1 term
    # u*w2c small. The per-l constant cdv ships exact as an f32 ACT bias.
    w2c = w2 - w2.mean()
    cdv = (c[:, D].astype(np.float64) - MBAR * w2c + SHIFT).astype(np.float32)

    cpk16 = np.zeros((128, CPK16_W), np.float16)
    for k in range(4):
        cpk16[:, P16_CT + k * 128 : P16_CT + (k + 1) * 128] = (
            c[:, k * 128 : (k + 1) * 128].T.astype(np.float16)
        )
    cpk16[0, P16_W2 : P16_W2 + 128] = (-w2c).astype(np.float16)

    cpk32 = np.zeros((128, CPK32_W), np.float32)
    cpk32[:, P32_IDENT : P32_IDENT + 128] = np.eye(128, dtype=np.float32)
    cpk32[:, P32_CDV] = cdv

    lpack = np.zeros((128, 2), np.uint32)
    lpack[:, LP_MASK] = np.uint32(ENC_MASK)
    lpack[:, LP_LIDX] = (127 - np.arange(128)).astype(np.uint32)
    return {"cpk16": cpk16, "cpk32": cpk32, "lpack": lpack}


def pack_feats_core(shard16):
    """[32768, 512] fp16 -> [128, 64, 2048]: P[p, g, k*512+n] = s[g*512+n, k*128+p]."""
    ngroups = shard16.shape[0] // GROUP
    P = shard16.reshape(ngroups, GROUP, 4, 128).transpose(3, 0, 2, 1)
    return np.ascontiguousarray(P).reshape(128, ngroups, 2048)


def build_in_maps(feats, initc, labelset):
    feats = np.asarray(feats, dtype=np.float32)
    consts = make_const_inputs(initc, labelset)

    feats16 = feats.astype(np.float16)
    q = np.einsum("nd,nd->n", feats, feats, dtype=np.float64)
    u16 = (np.sqrt(q + 1.0) - MBAR).astype(np.float16)

    in_maps = []
    for core in range(N_CORES):
        sl = slice(core * ROWS_PER_CORE, (core + 1) * ROWS_PER_CORE)
        in_maps.append(
            {
                "ftpack": pack_feats_core(feats16[sl]),
                "upack": np.ascontiguousarray(u16[sl].reshape(1, -1)),
                **consts,
            }
        )
    return in_maps


def kernel(feats, initc, labelset):
    labelset_np = np.asarray(labelset)
    in_maps = build_in_maps(feats, initc, labelset)
    nc = build_core_program(ROWS_PER_CORE)
    res = run_bass_kernel_spmd(nc, in_maps, list(range(N_CORES)))

    preds = []
    for core in range(N_CORES):
        enc = np.asarray(res.results[core]["idx8"])  # [128, ngroups, 4] u32
        idx = (127 - (enc & np.uint32(127))).astype(np.int64)
        # [p, g, j] -> row 512*g + 128*j + p
        preds.append(idx.transpose(1, 2, 0).reshape(-1))
    idx_all = np.concatenate(preds)
    return labelset_np[idx_all]
